# revision 1
# baseline (speedup 1.0000x reference)
"""Trainium2 Bass kernel for nn_MultiHeadAttention_65987877535893.

MHA with RoPE: B=2, S=2048, D=1024, H=16, Dh=64, causal mask.

Sharding (8 cores): data-parallel over B (x2) x tensor-parallel over heads
(x4 -> 4 heads/core).  Each core computes, for its batch b and head group g:
  QKV projections (column-sharded W), RoPE, causal attention, and a partial
  output  A_g @ Wo_g  (row-sharded Wo).  Host sums the 4 partials per batch.

Device algorithm (per core), all matmuls in float32r (full PE rate):
  - xT [D,S] streamed in 512-col slices; Q/K projected directly into
    "head-transposed" layout via lhsT=W chunks; RoPE on [128,512] tiles in
    x1-stacked layout, repacked into head-contiguous per-slice tiles
    qr/kr [128, 512] (2 heads x 64 dims on partitions).
  - scores computed TRANSPOSED: ST[k,q] = K @ Q^T per (head, k-block of 128,
    q-macro of 512); exp on ScalarE (scale=1/8 folded in, exact causal
    widths); causal masking by skipping upper blocks + multiplying the
    diagonal macro by a binary mask tile; probs stay transposed so P^T
    feeds PV directly.
  - PV: lhsT=[V_h | ones] [128,65] -> A^T[64,q] plus the softmax denominator
    row for free; normalize with reciprocal(rank-1-broadcast sums).
  - output: O[s,:] = A^T.T @ Wo chunks, PSUM->SBUF->DRAM.
  All tensors are split into per-512-slice tiles so Tile's tile-granular
  dependency tracking lets attention start while projections still run.
"""

import os
import sys

sys.path.insert(0, "/opt/trn_rl_repo")
os.environ.setdefault("MYCRO_LOCAL_CACHE", "1")

import numpy as np

import concourse.bass as bass
import concourse.bacc as bacc
import concourse.mybir as mybir
import concourse.tile as tile
from concourse.bass_utils import run_bass_kernel_spmd

F32 = mybir.dt.float32
F32R = mybir.dt.float32r

B, S, D = 2, 2048, 1024
H = 16
DH = 64
HPC = 4           # heads per core
DG = HPC * DH     # 256
N_CORES = 8
KO = D // 128     # 8 contraction chunks
N_SLICES = S // 512   # 4 q/s slices
N_KB = S // 128       # 16 k-blocks
EXP_SCALE = float(DH) ** -0.5  # 0.125
Exp = mybir.ActivationFunctionType.Exp


def r(ap):
    return ap.bitcast(F32R)


def build_nc():
    nc = bacc.Bacc()

    xT = nc.dram_tensor("xT", [D, S], F32, kind="ExternalInput")
    wq1 = nc.dram_tensor("wq1", [D, 128], F32, kind="ExternalInput")
    wq2 = nc.dram_tensor("wq2", [D, 128], F32, kind="ExternalInput")
    wk1 = nc.dram_tensor("wk1", [D, 128], F32, kind="ExternalInput")
    wk2 = nc.dram_tensor("wk2", [D, 128], F32, kind="ExternalInput")
    wv = nc.dram_tensor("wv", [D, DG], F32, kind="ExternalInput")
    wo = nc.dram_tensor("wo", [DG, D], F32, kind="ExternalInput")
    ccT = nc.dram_tensor("ccT", [128, S], F32, kind="ExternalInput")
    ssT = nc.dram_tensor("ssT", [128, S], F32, kind="ExternalInput")
    bmask = nc.dram_tensor("bmask", [128, 4, 512], F32, kind="ExternalInput")
    o_part = nc.dram_tensor("o_part", [S, D], F32, kind="ExternalOutput")

    with tile.TileContext(nc) as tc:
        import contextlib

        ctx = contextlib.ExitStack()
        with ctx:
            persist = ctx.enter_context(tc.tile_pool(name="persist", bufs=1))
            work = ctx.enter_context(tc.tile_pool(name="work", bufs=2))

            # ---- persistent SBUF tensors (per-slice granularity) ----
            qr = [[persist.tile([128, 512], F32R, tag=f"qr{p}_{ms}", name=f"qr{p}_{ms}")
                   for ms in range(N_SLICES)] for p in range(2)]
            kr = [[persist.tile([128, 512], F32R, tag=f"kr{p}_{ms}", name=f"kr{p}_{ms}")
                   for ms in range(N_SLICES)] for p in range(2)]
            # V with a ones column per (kb, head): [128, kb_in_slice, head, 65]
            vt = [persist.tile([128, 4, HPC, DH + 1], F32R, tag=f"vt{ms}", name=f"vt{ms}")
                  for ms in range(N_SLICES)]
            atn = [[persist.tile([128, 512], F32R, tag=f"atn{p}_{ms}", name=f"atn{p}_{ms}")
                    for ms in range(N_SLICES)] for p in range(2)]
            cc_sb = persist.tile([128, S], F32, tag="cc", name="cc")
            ss_sb = persist.tile([128, S], F32, tag="ss", name="ss")
            bm_sb = persist.tile([128, 4, 512], F32R, tag="bm", name="bm")
            wq1_sb = persist.tile([128, KO, 128], F32R, tag="wq1", name="wq1")
            wq2_sb = persist.tile([128, KO, 128], F32R, tag="wq2", name="wq2")
            wk1_sb = persist.tile([128, KO, 128], F32R, tag="wk1", name="wk1")
            wk2_sb = persist.tile([128, KO, 128], F32R, tag="wk2", name="wk2")
            wv_sb = persist.tile([128, KO, DG], F32R, tag="wv", name="wv")
            wo_sb = persist.tile([128, 2, D], F32R, tag="wo", name="wo")

            for w_sb, w_dr in ((wq1_sb, wq1), (wq2_sb, wq2), (wk1_sb, wk1),
                               (wk2_sb, wk2)):
                nc.scalar.dma_start(
                    w_sb[:], w_dr.rearrange("(ko p) m -> p ko m", p=128).bitcast(F32R))
            nc.scalar.dma_start(cc_sb[:], ccT[:])
            nc.scalar.dma_start(ss_sb[:], ssT[:])
            nc.scalar.dma_start(
                wv_sb[:], wv.rearrange("(ko p) m -> p ko m", p=128).bitcast(F32R))
            nc.scalar.dma_start(bm_sb[:], bmask[:].bitcast(F32R))
            nc.scalar.dma_start(
                wo_sb[:], wo.rearrange("(ko p) m -> p ko m", p=128).bitcast(F32R))

            onesf = persist.tile([128, 64], F32, tag="onesf", name="onesf")
            ones1 = persist.tile([1, 64], F32R, tag="ones1", name="ones1")
            nc.vector.memset(onesf[:], 1.0)
            nc.vector.tensor_copy(ones1[:], onesf[0:1, :])
            for ms in range(N_SLICES):
                nc.vector.tensor_copy(
                    vt[ms][:, :, :, DH],
                    onesf[:, 0:16].rearrange("p (a b) -> p a b", b=HPC))

            # ================= projections + rope + V =================
            with tc.tile_pool(name="projps", bufs=6, space="PSUM") as projps, \
                 tc.tile_pool(name="vps", bufs=2, space="PSUM") as vps:
                for m in range(N_SLICES):
                    sl = slice(512 * m, 512 * (m + 1))
                    xts = work.tile([128, KO, 512], F32R, tag="xts", name="xts")
                    xTr = xT.rearrange("(ko p) s -> p ko s", p=128)
                    nc.sync.dma_start(xts[:, 0:4], xTr[:, 0:4, sl].bitcast(F32R))
                    nc.sync.dma_start(xts[:, 4:8], xTr[:, 4:8, sl].bitcast(F32R))

                    for (w1_sb, w2_sb, dst) in ((wq1_sb, wq2_sb, qr),
                                                (wk1_sb, wk2_sb, kr)):
                        p1 = projps.tile([128, 512], F32, tag="pp", name="pp")
                        p2 = projps.tile([128, 512], F32, tag="pp", name="pp")
                        for ko in range(KO):
                            nc.tensor.matmul(p1[:], r(w1_sb[:, ko]), r(xts[:, ko]),
                                             start=(ko == 0), stop=(ko == KO - 1))
                        for ko in range(KO):
                            nc.tensor.matmul(p2[:], r(w2_sb[:, ko]), r(xts[:, ko]),
                                             start=(ko == 0), stop=(ko == KO - 1))
                        # rope on x1-stacked [128,512] tiles
                        t1 = work.tile([128, 512], F32, tag="t1", name="t1")
                        t2 = work.tile([128, 512], F32, tag="t2", name="t2")
                        t3 = work.tile([128, 512], F32, tag="t3", name="t3")
                        t4 = work.tile([128, 512], F32, tag="t4", name="t4")
                        nc.vector.tensor_mul(t1[:], p1[:], cc_sb[:, sl])
                        nc.vector.tensor_mul(t2[:], p2[:], ss_sb[:, sl])
                        nc.vector.tensor_mul(t3[:], p2[:], cc_sb[:, sl])
                        nc.vector.tensor_mul(t4[:], p1[:], ss_sb[:, sl])
                        for h in range(HPC):
                            pr, a = h // 2, h % 2
                            hs = slice(32 * h, 32 * h + 32)
                            nc.vector.tensor_sub(
                                dst[pr][m][64 * a:64 * a + 32, :], t1[hs], t2[hs])
                            nc.vector.tensor_add(
                                dst[pr][m][64 * a + 32:64 * a + 64, :], t3[hs], t4[hs])

                    for sc in range(4):
                        pv = vps.tile([128, DG], F32, tag="vp", name="vp")
                        for ko in range(KO):
                            nc.tensor.matmul(
                                pv[:], r(xts[:, ko, 128 * sc:128 * sc + 128]),
                                r(wv_sb[:, ko]),
                                start=(ko == 0), stop=(ko == KO - 1))
                        nc.scalar.copy(
                            vt[m][:, sc, :, 0:DH],
                            pv[:].rearrange("p (h d) -> p h d", d=DH))

            # ================= attention + output =================
            with tc.tile_pool(name="stps", bufs=2, space="PSUM") as stps, \
                 tc.tile_pool(name="atps", bufs=1, space="PSUM") as atps, \
                 tc.tile_pool(name="wops", bufs=1, space="PSUM") as wops:
                for m in range(N_SLICES):
                    for p in range(2):
                        at = [atps.tile([DH + 1, 512], F32, tag=f"at{a}",
                                        name=f"at{a}") for a in range(2)]
                        for kb in range(4 * m + 4):
                            st = stps.tile([128, 2, 512], F32, tag="st", name="st")
                            ksl = slice(128 * (kb % 4), 128 * (kb % 4) + 128)
                            diag = (kb // 4 == m)
                            c0 = 128 * (kb % 4) if diag else 0
                            for a in range(2):
                                nc.tensor.matmul(
                                    st[:, a, c0:],
                                    r(kr[p][kb // 4][64 * a:64 * a + 64, ksl]),
                                    r(qr[p][m][64 * a:64 * a + 64, c0:]),
                                    start=True, stop=True)
                            pt = work.tile([128, 2, 512], F32R, tag="pt", name="pt")
                            nc.scalar.activation(
                                pt[:, :, c0:], st[:, :, c0:], Exp, scale=EXP_SCALE)
                            if diag:
                                j = kb % 4
                                nc.vector.tensor_mul(
                                    pt[:, :, c0:], pt[:, :, c0:],
                                    bm_sb[:, j:j + 1, c0:].to_broadcast(
                                        (128, 2, 512 - c0)))
                            for a in range(2):
                                nc.tensor.matmul(
                                    at[a][:, c0:], r(vt[kb // 4][:, kb % 4, 2 * p + a]),
                                    r(pt[:, a, c0:]),
                                    start=(kb == 0), stop=(kb == 4 * m + 3))
                        # normalize: ATn = at[0:64] * recip(bcast(at[64]))
                        for a in range(2):
                            ssum = work.tile([1, 512], F32R, tag="ssum", name="ssum")
                            rbc = work.tile([64, 512], F32, tag="rbc", name="rbc")
                            nc.vector.tensor_copy(ssum[:], at[a][DH:DH + 1, :])
                            sbc = wops.tile([64, 512], F32, tag="wop0", name="sbc")
                            nc.tensor.matmul(sbc[:], r(ones1), r(ssum[:]),
                                             start=True, stop=True)
                            nc.vector.reciprocal_approx_fast(rbc[:], sbc[:])
                            nc.vector.tensor_mul(
                                atn[p][m][64 * a:64 * a + 64, :], at[a][0:DH, :],
                                rbc[:])
                    # output projection for the 4 s-chunks of this macro
                    for sc in range(4 * m, 4 * m + 4):
                        scl = slice(128 * (sc % 4), 128 * (sc % 4) + 128)
                        osb = work.tile([128, D], F32, tag="osb", name="osb")
                        po = [wops.tile([128, 512], F32, tag=f"wop{nh}",
                                        name=f"wop{nh}") for nh in range(2)]
                        for ksub in range(2):
                            for nh in range(2):
                                nsl = slice(512 * nh, 512 * (nh + 1))
                                nc.tensor.matmul(
                                    po[nh][:], r(atn[ksub][sc // 4][:, scl]),
                                    r(wo_sb[:, ksub, nsl]),
                                    start=(ksub == 0), stop=(ksub == 1))
                        for nh in range(2):
                            nc.vector.tensor_copy(
                                osb[:, 512 * nh:512 * nh + 512], po[nh][:])
                        nc.sync.dma_start(o_part[128 * sc:128 * sc + 128, :], osb[:])

    nc.finalize()
    return nc


def prep_inputs(hidden_states, cos, sin, attention_mask, Wq, Wk, Wv, Wo):
    """Host-side sharding/layout prep. Returns in_maps for the 8 cores."""
    hs = np.asarray(hidden_states, dtype=np.float32)
    cos = np.asarray(cos, dtype=np.float32)
    sin = np.asarray(sin, dtype=np.float32)
    mask = np.asarray(attention_mask, dtype=np.float32)
    Wq = np.asarray(Wq, dtype=np.float32)
    Wk = np.asarray(Wk, dtype=np.float32)
    Wv = np.asarray(Wv, dtype=np.float32)
    Wo = np.asarray(Wo, dtype=np.float32)

    ccT = np.ascontiguousarray(np.tile(cos.T, (4, 1)))  # [128, S]
    ssT = np.ascontiguousarray(np.tile(sin.T, (4, 1)))

    # binary mask tiles [128, 4, 512]: bmask[kappa, j, phi] = allowed(q=phi, k=128j+kappa)
    mblock = mask[0, 0, 0:512, 0:512]  # [q, k]
    bm = np.empty((128, 4, 512), dtype=np.float32)
    for j in range(4):
        bm[:, j, :] = (mblock[:, 128 * j:128 * j + 128] >= -0.5).T.astype(np.float32)
    bm = np.ascontiguousarray(bm)

    xTs = [np.ascontiguousarray(hs[b].T) for b in range(B)]

    in_maps = []
    for c in range(N_CORES):
        b, g = c // 4, c % 4
        hsl = slice(DG * g, DG * (g + 1))
        wq_g = Wq[:, hsl].reshape(D, HPC, DH)
        wk_g = Wk[:, hsl].reshape(D, HPC, DH)
        in_maps.append({
            "xT": xTs[b],
            "wq1": np.ascontiguousarray(wq_g[:, :, :32].reshape(D, 128)),
            "wq2": np.ascontiguousarray(wq_g[:, :, 32:].reshape(D, 128)),
            "wk1": np.ascontiguousarray(wk_g[:, :, :32].reshape(D, 128)),
            "wk2": np.ascontiguousarray(wk_g[:, :, 32:].reshape(D, 128)),
            "wv": np.ascontiguousarray(Wv[:, hsl]),
            "wo": np.ascontiguousarray(Wo[hsl, :]),
            "ccT": ccT,
            "ssT": ssT,
            "bmask": bm,
        })
    return in_maps


_NC_CACHE = {}


def get_nc():
    if "nc" not in _NC_CACHE:
        _NC_CACHE["nc"] = build_nc()
    return _NC_CACHE["nc"]


def run(inputs, trace=False):
    """Returns (output [B,S,D] fp32, BassKernelResults)."""
    nc = get_nc()
    in_maps = prep_inputs(**inputs)
    res = run_bass_kernel_spmd(nc, in_maps, list(range(N_CORES)), trace=trace)
    out = np.zeros((B, S, D), dtype=np.float32)
    for c in range(N_CORES):
        out[c // 4] += res.results[c]["o_part"]
    return out, res


def kernel(**inputs):
    return run(inputs, trace=False)[0]



# revision 9
# speedup vs baseline: 1.4307x; 1.4307x over previous
"""Trainium2 Bass kernel for nn_MultiHeadAttention_65987877535893.

MHA with RoPE: B=2, S=2048, D=1024, H=16, Dh=64, causal mask.

Sharding (8 cores): data-parallel over B (x2) x tensor-parallel over heads
(x4 -> 4 heads/core).  Each core computes, for its batch b and head group g:
  QKV projections (column-sharded W), RoPE, causal attention, and a partial
  output  A_g @ Wo_g  (row-sharded Wo).  Host sums the 4 partials per batch.

v2.2 (bf16, software-pipelined): all matmul inputs bf16 (PSUM accum fp32).
 - Wq/Wk columns host-permuted to pair-interleaved order per head (x1_i at
   partition 2i, x2_i at 2i+1), so RoPE is 4 full-width [128,512] DVE ops:
   r1 = p*cc, r2 = p*ss (cross-term sign baked into ss), r2s = pairswap(r2)
   via stream_shuffle, q = r1 + r2s.
 - Causal masking of diagonal blocks via gpsimd affine_select
   (iota = col - kappa >= 0) on the exp'd probs -- no mask tensor.
 - Scores emitted one kb ahead of PV; projection work for slice m+1 is
   emitted in chunks interleaved into macro m's attention stream so the PE
   never drains (keeps HAM at full clock); p=0 normalize deferred past the
   next group's first scores.
 - PSUM: pp(proj/sbc/outproj) 2 banks + st 4 banks + at 2 banks = 8.
"""

import os
import sys

sys.path.insert(0, "/opt/trn_rl_repo")
os.environ.setdefault("MYCRO_LOCAL_CACHE", "1")

import numpy as np

import concourse.bass as bass
import concourse.bacc as bacc
import concourse.mybir as mybir
import concourse.tile as tile
from concourse.bass_utils import run_bass_kernel_spmd

F32 = mybir.dt.float32
BF16 = mybir.dt.bfloat16

B, S, D = 2, 2048, 1024
H = 16
DH = 64
HPC = 4           # heads per core
DG = HPC * DH     # 256
N_CORES = 8
KO = D // 128     # 8 contraction chunks
N_SLICES = S // 512   # 4 q/s slices
EXP_SCALE = float(DH) ** -0.5  # 0.125
Exp = mybir.ActivationFunctionType.Exp
GE = mybir.AluOpType.is_ge

# pair-swap within quadrants: 0<->1, 2<->3, ...
SWAP_MASK = [i ^ 1 for i in range(32)]


def build_nc():
    nc = bacc.Bacc()

    xT = nc.dram_tensor("xT", [D, S], BF16, kind="ExternalInput")
    wqa = nc.dram_tensor("wqa", [D, 128], BF16, kind="ExternalInput")
    wqb = nc.dram_tensor("wqb", [D, 128], BF16, kind="ExternalInput")
    wka = nc.dram_tensor("wka", [D, 128], BF16, kind="ExternalInput")
    wkb = nc.dram_tensor("wkb", [D, 128], BF16, kind="ExternalInput")
    wv = nc.dram_tensor("wv", [D, DG], BF16, kind="ExternalInput")
    wo = nc.dram_tensor("wo", [DG, D], BF16, kind="ExternalInput")
    ccT = nc.dram_tensor("ccT", [128, S], F32, kind="ExternalInput")
    ssT = nc.dram_tensor("ssT", [128, S], F32, kind="ExternalInput")
    o_part = nc.dram_tensor("o_part", [S, D], BF16, kind="ExternalOutput")

    with tile.TileContext(nc) as tc:
        import contextlib

        ctx = contextlib.ExitStack()
        with ctx:
            persist = ctx.enter_context(tc.tile_pool(name="persist", bufs=1))
            work = ctx.enter_context(tc.tile_pool(name="work", bufs=2))

            # ---- persistent SBUF tensors ----
            qr = [[persist.tile([128, 512], BF16, tag=f"qr{p}_{ms}", name=f"qr{p}_{ms}")
                   for ms in range(N_SLICES)] for p in range(2)]
            kr = [[persist.tile([128, 512], BF16, tag=f"kr{p}_{ms}", name=f"kr{p}_{ms}")
                   for ms in range(N_SLICES)] for p in range(2)]
            vt = [persist.tile([128, 4, HPC, DH + 1], BF16, tag=f"vt{ms}", name=f"vt{ms}")
                  for ms in range(N_SLICES)]
            atn = [[persist.tile([128, 512], BF16, tag=f"atn{p}_{ms}", name=f"atn{p}_{ms}")
                    for ms in range(N_SLICES)] for p in range(2)]
            cc_sb = persist.tile([128, S], F32, tag="cc", name="cc")
            ss_sb = persist.tile([128, S], F32, tag="ss", name="ss")
            wqa_sb = persist.tile([128, KO, 128], BF16, tag="wqa", name="wqa")
            wqb_sb = persist.tile([128, KO, 128], BF16, tag="wqb", name="wqb")
            wka_sb = persist.tile([128, KO, 128], BF16, tag="wka", name="wka")
            wkb_sb = persist.tile([128, KO, 128], BF16, tag="wkb", name="wkb")
            wv_sb = persist.tile([128, KO, DG], BF16, tag="wv", name="wv")
            wo_sb = persist.tile([128, 2, D], BF16, tag="wo", name="wo")

            # xts for slice 0 first so the first proj can start ASAP
            xTr = xT.rearrange("(ko p) s -> p ko s", p=128)
            xts0 = work.tile([128, KO, 512], BF16, tag="xts", name="xts0")
            nc.sync.dma_start(xts0[:, 0:4], xTr[:, 0:4, 0:512])
            nc.sync.dma_start(xts0[:, 4:8], xTr[:, 4:8, 0:512])

            # weights on the scalar queue, in first-use order
            for w_sb, w_dr in ((wqa_sb, wqa), (wka_sb, wka)):
                nc.scalar.dma_start(
                    w_sb[:], w_dr.rearrange("(ko p) m -> p ko m", p=128))
            nc.scalar.dma_start(
                wv_sb[:], wv.rearrange("(ko p) m -> p ko m", p=128))
            for w_sb, w_dr in ((wqb_sb, wqb), (wkb_sb, wkb)):
                nc.scalar.dma_start(
                    w_sb[:], w_dr.rearrange("(ko p) m -> p ko m", p=128))
            nc.scalar.dma_start(
                wo_sb[:], wo.rearrange("(ko p) m -> p ko m", p=128))
            # rope tables on the gpsimd queue (idle at start)
            nc.gpsimd.dma_start(cc_sb[:], ccT[:])
            nc.gpsimd.dma_start(ss_sb[:], ssT[:])

            onesf = persist.tile([128, 16], F32, tag="onesf", name="onesf")
            ones1 = persist.tile([1, 64], BF16, tag="ones1", name="ones1")
            nc.vector.memset(onesf[:], 1.0)
            nc.vector.memset(ones1[:], 1.0)
            for ms in range(N_SLICES):
                nc.vector.tensor_copy(
                    vt[ms][:, :, :, DH],
                    onesf[:, 0:16].rearrange("p (a b) -> p a b", b=HPC))

            # PSUM pools: pp 2 banks + st 4 banks + at 2 banks = 8
            pp = ctx.enter_context(tc.tile_pool(name="pp", bufs=2, space="PSUM"))
            stp = ctx.enter_context(tc.tile_pool(name="stp", bufs=2, space="PSUM"))
            atp = ctx.enter_context(tc.tile_pool(name="atp", bufs=1, space="PSUM"))

            # ---------------- projection chunk emitters ----------------
            xts_cell = {0: xts0}

            def c_dma(m):
                def f():
                    xts = work.tile([128, KO, 512], BF16, tag="xts",
                                    name=f"xts{m}")
                    xts_cell[m] = xts
                    sl = slice(512 * m, 512 * (m + 1))
                    nc.sync.dma_start(xts[:, 0:4], xTr[:, 0:4, sl])
                    nc.sync.dma_start(xts[:, 4:8], xTr[:, 4:8, sl])
                return f

            def c_qk(m, w_sb, dst):
                def f():
                    xts = xts_cell[m]
                    sl = slice(512 * m, 512 * (m + 1))
                    pj = pp.tile([128, 512], F32, tag="pp", name="pj")
                    for ko in range(KO):
                        nc.tensor.matmul(pj[:], w_sb[:, ko], xts[:, ko],
                                         start=(ko == 0), stop=(ko == KO - 1))
                    r1 = work.tile([128, 512], BF16, tag="r1", name="r1")
                    r2 = work.tile([128, 512], BF16, tag="r2", name="r2")
                    r2s = work.tile([128, 512], BF16, tag="r2s", name="r2s")
                    nc.vector.tensor_mul(r1[:], pj[:], cc_sb[:, sl])
                    nc.vector.tensor_mul(r2[:], pj[:], ss_sb[:, sl])
                    nc.vector.stream_shuffle(r2s[:], r2[:], SWAP_MASK)
                    nc.vector.tensor_add(dst[m][:], r1[:], r2s[:])
                return f

            def c_v(m, half):
                def f():
                    xts = xts_cell[m]
                    pv = pp.tile([128, 512], F32, tag="pp", name="pv")
                    for sc in range(2):
                        xsl = slice(128 * (2 * half + sc),
                                    128 * (2 * half + sc) + 128)
                        for ko in range(KO):
                            nc.tensor.matmul(
                                pv[:, 256 * sc:256 * sc + 256],
                                xts[:, ko, xsl], wv_sb[:, ko],
                                start=(ko == 0), stop=(ko == KO - 1))
                    nc.vector.tensor_copy(
                        vt[m][:, 2 * half:2 * half + 2, :, 0:DH],
                        pv[:].rearrange("p (sc h d) -> p sc h d", sc=2, d=DH))
                return f

            def proj_chunks(m):
                return [c_dma(m),
                        c_qk(m, wqa_sb, qr[0]), c_qk(m, wka_sb, kr[0]),
                        c_v(m, 0), c_v(m, 1),
                        c_qk(m, wqb_sb, qr[1]), c_qk(m, wkb_sb, kr[1])]

            # slice 0 (xts0 already DMA'd above): emit what macro-0 p=0
            # needs up front; QB/KB interleave into the p=0 stream
            for c in proj_chunks(0)[1:5]:
                c()
            pending = proj_chunks(0)[5:]
            need_before_p1 = len(pending)   # QB0/KB0 must precede p=1 scores

            # ---------------- attention ----------------
            def normalize(p, m, at):
                def f():
                    for a in range(2):
                        ssb = work.tile([1, 512], BF16, tag="ssb", name="ssb")
                        rbc = work.tile([64, 512], F32, tag="rbc", name="rbc")
                        nc.vector.tensor_copy(ssb[:], at[a][DH:DH + 1, :])
                        sbc = pp.tile([128, 512], F32, tag="pp", name="sbc")
                        nc.tensor.matmul(sbc[0:64, :], ones1, ssb[:],
                                         start=True, stop=True)
                        nc.vector.reciprocal_approx_fast(rbc[:], sbc[0:64, :])
                        nc.vector.tensor_mul(
                            atn[p][m][64 * a:64 * a + 64, :], at[a][0:DH, :],
                            rbc[:])
                return f

            def outproj_chunk(m, sc):
                def f():
                    scl = slice(128 * (sc % 4), 128 * (sc % 4) + 128)
                    osb = work.tile([128, D], BF16, tag="osb", name="osb")
                    po = [pp.tile([128, 512], F32, tag="pp", name=f"po{nh}")
                          for nh in range(2)]
                    for nh in range(2):
                        for ksub in range(2):
                            nc.tensor.matmul(
                                po[nh][:], atn[ksub][m][:, scl],
                                wo_sb[:, ksub, 512 * nh:512 * nh + 512],
                                start=(ksub == 0), stop=(ksub == 1))
                    for nh in range(2):
                        nc.vector.tensor_copy(
                            osb[:, 512 * nh:512 * nh + 512], po[nh][:])
                    nc.sync.dma_start(o_part[128 * sc:128 * sc + 128, :], osb[:])
                return f

            deferred = [None]   # normalize of the previous (m, p) group

            for m in range(N_SLICES):
                if m > 0:
                    pending = [outproj_chunk(m - 1, sc)
                               for sc in range(4 * (m - 1), 4 * (m - 1) + 4)]
                    need_before_p1 = 0
                if m + 1 < N_SLICES:
                    pending.extend(proj_chunks(m + 1))
                nkb = 4 * m + 4
                iters = 2 * nkb
                it = 0
                emitted = 0
                for p in range(2):
                    sts = {}

                    def emit_scores(kb, p=p, m=m, sts=sts):
                        st = stp.tile([128, 2, 512], F32, tag="st", name="st")
                        sts[kb] = st
                        ksl = slice(128 * (kb % 4), 128 * (kb % 4) + 128)
                        c0 = 128 * (kb % 4) if (kb // 4 == m) else 0
                        for a in range(2):
                            nc.tensor.matmul(
                                st[:, a, c0:],
                                kr[p][kb // 4][64 * a:64 * a + 64, ksl],
                                qr[p][m][64 * a:64 * a + 64, c0:],
                                start=True, stop=True)

                    if p == 1:
                        while emitted < need_before_p1:
                            pending[emitted]()
                            emitted += 1
                    emit_scores(0)
                    # flush the previous group's normalize now, before the
                    # at-pool slots are reallocated below (WAR ordering)
                    if deferred[0] is not None:
                        deferred[0]()
                        deferred[0] = None
                    at = [atp.tile([DH + 1, 512], F32, tag=f"at{a}",
                                   name=f"at{a}") for a in range(2)]
                    for kb in range(nkb):
                        diag = (kb // 4 == m)
                        c0 = 128 * (kb % 4) if diag else 0
                        st = sts.pop(kb)
                        pt = work.tile([128, 2, 512], BF16, tag="pt", name="pt",
                                       bufs=3)
                        nc.scalar.activation(
                            pt[:, :, c0:], st[:, :, c0:], Exp, scale=EXP_SCALE)
                        if diag:
                            w = 512 - c0
                            nc.gpsimd.affine_select(
                                pt[:, :, c0:], pt[:, :, c0:],
                                pattern=[[0, 2], [1, w]],
                                compare_op=GE, fill=0.0,
                                base=0, channel_multiplier=-1)
                        if kb + 1 < nkb:
                            emit_scores(kb + 1)
                        for a in range(2):
                            nc.tensor.matmul(
                                at[a][:, c0:], vt[kb // 4][:, kb % 4, 2 * p + a],
                                pt[:, a, c0:],
                                start=(kb == 0), stop=(kb == nkb - 1))
                        # interleave pending chunks (outproj m-1, proj m+1)
                        it += 1
                        want = (it * len(pending)) // max(iters, 1)
                        while emitted < want and emitted < len(pending):
                            pending[emitted]()
                            emitted += 1

                    deferred[0] = normalize(p, m, at)
                while emitted < len(pending):
                    pending[emitted]()
                    emitted += 1
                pending = []

            # tail: last normalize + last macro's output projection
            deferred[0]()
            for sc in range(4 * (N_SLICES - 1), 4 * N_SLICES):
                outproj_chunk(N_SLICES - 1, sc)()

    nc.finalize()
    return nc


def _to_bf16(x):
    from ml_dtypes import bfloat16
    return np.asarray(x, dtype=np.float32).astype(bfloat16)


def prep_inputs(hidden_states, cos, sin, attention_mask, Wq, Wk, Wv, Wo):
    """Host-side sharding/layout prep. Returns in_maps for the 8 cores."""
    hs = np.asarray(hidden_states, dtype=np.float32)
    cos = np.asarray(cos, dtype=np.float32)
    sin = np.asarray(sin, dtype=np.float32)
    Wq = np.asarray(Wq, dtype=np.float32)
    Wk = np.asarray(Wk, dtype=np.float32)
    Wv = np.asarray(Wv, dtype=np.float32)
    Wo = np.asarray(Wo, dtype=np.float32)

    # rope tables in pair-interleaved layout, 2 heads (128 partitions) per tile
    idx = np.empty(64, dtype=np.int64)
    idx[0::2] = np.arange(32)
    idx[1::2] = np.arange(32)
    cc1 = cos.T[idx]                      # [64, S]
    ss1 = sin.T[idx].copy()               # [64, S]
    ss1[1::2] *= -1.0
    ccT = np.ascontiguousarray(np.tile(cc1, (2, 1)), dtype=np.float32)  # [128,S]
    ssT = np.ascontiguousarray(np.tile(ss1, (2, 1)), dtype=np.float32)

    # per-head column permutation of Wq/Wk into interleaved order
    perm = np.empty(64, dtype=np.int64)
    perm[0::2] = np.arange(32)        # x1_i = dim i
    perm[1::2] = np.arange(32) + 32   # x2_i = dim 32+i

    xTs = [np.ascontiguousarray(_to_bf16(hs[b].T)) for b in range(B)]

    in_maps = []
    for c in range(N_CORES):
        b, g = c // 4, c % 4
        hsl = slice(DG * g, DG * (g + 1))
        wq_g = Wq[:, hsl].reshape(D, HPC, DH)[:, :, perm]   # [D, 4, 64]
        wk_g = Wk[:, hsl].reshape(D, HPC, DH)[:, :, perm]
        in_maps.append({
            "xT": xTs[b],
            "wqa": np.ascontiguousarray(_to_bf16(wq_g[:, 0:2].reshape(D, 128))),
            "wqb": np.ascontiguousarray(_to_bf16(wq_g[:, 2:4].reshape(D, 128))),
            "wka": np.ascontiguousarray(_to_bf16(wk_g[:, 0:2].reshape(D, 128))),
            "wkb": np.ascontiguousarray(_to_bf16(wk_g[:, 2:4].reshape(D, 128))),
            "wv": np.ascontiguousarray(_to_bf16(Wv[:, hsl])),
            "wo": np.ascontiguousarray(_to_bf16(Wo[hsl, :])),
            "ccT": ccT,
            "ssT": ssT,
        })
    return in_maps


_NC_CACHE = {}


def get_nc():
    if "nc" not in _NC_CACHE:
        _NC_CACHE["nc"] = build_nc()
    return _NC_CACHE["nc"]


def run(inputs, trace=False):
    """Returns (output [B,S,D] fp32, BassKernelResults)."""
    nc = get_nc()
    in_maps = prep_inputs(**inputs)
    res = run_bass_kernel_spmd(nc, in_maps, list(range(N_CORES)), trace=trace)
    out = np.zeros((B, S, D), dtype=np.float32)
    for c in range(N_CORES):
        out[c // 4] += np.asarray(res.results[c]["o_part"], dtype=np.float32)
    return out, res


def kernel(**inputs):
    return run(inputs, trace=False)[0]
